# revision 14
# baseline (speedup 1.0000x reference)
"""Trainium2 Bass kernel for EMA-along-L + residual, x: (32, 4096, 512) fp32.

Blocked-matmul EMA formulation, fp16 I/O.

With alpha=0.3 (beta=0.7), beta^128 ~ 1.6e-20, so the EMA state at step t
depends only on the last <=256 inputs to far below fp32 precision. For a
128-row L-chunk X_c (layout [l=partition, d=free] -- the natural, cheap DMA
layout), the whole chunk's EMA is

    ma_c = M_low @ X_c + M_high @ X_{c-1}

with constant 128x128 matrices M_low[t,j] = alpha*beta^(t-j) (j<=t) and
M_high[t,j] = alpha*beta^(t+128-j); the first chunk of each batch instead
uses M_first (column 0 replaced by beta^t to match the s_0 = x_0 seed).
That is two PE matmuls per chunk with constant stationary weights, no
transposes and no serial carry chain at all.

I/O runs in fp16 (inputs downcast / outputs upcast on host): halves HBM
traffic vs fp32; end-to-end error ~6e-4 vs the 2e-2 gate. Per chunk, ACT
casts PSUM->fp16 ma tile and DVE computes res = x - ma (fp16 out). Loads
are issued on the Sync queue, stores on the GpSimd queue, so input
prefetch never queues behind a store's semaphore wait.

Sharding: batch dim (32) split 4-per-core across 8 NeuronCores; the scan
dim L stays on-core, so no cross-device communication.
"""

import sys

import numpy as np

try:
    import concourse.bass as bass  # noqa: F401
except ImportError:
    sys.path.insert(0, "/opt/trn_rl_repo")

import concourse.bacc as bacc
import concourse.bass as bass
import concourse.mybir as mybir
import concourse.tile as tile
from concourse.bass_utils import run_bass_kernel_spmd

ALPHA = 0.3
BETA = 0.7

B, L, D = 32, 4096, 512
NCORES = 8
BLOC = B // NCORES  # 4 batches per core
T = 128  # chunk rows (PE contraction size)
G = 8  # chunks per DMA group
LG = T * G  # 1024 rows per group
NG = L // LG  # 4 groups per batch

_F16 = mybir.dt.float16
_F32 = mybir.dt.float32


def _weights():
    """W_first/W_low/W_high, pre-transposed to [j, t] for the PE lhsT slot."""
    ti = np.arange(T)
    p = ti[:, None] - ti[None, :]
    with np.errstate(under="ignore"):
        m_low = np.where(p >= 0, ALPHA * BETA ** np.clip(p, 0, None), 0.0)
        m_first = m_low.copy()
        m_first[:, 0] = BETA**ti
        m_high = ALPHA * BETA ** (ti[:, None] + T - ti[None, :])
    return (
        np.ascontiguousarray(m_first.T).astype(np.float16),
        np.ascontiguousarray(m_low.T).astype(np.float16),
        np.ascontiguousarray(m_high.T).astype(np.float16),
    )


_NC_CACHE = None


def build():
    global _NC_CACHE
    if _NC_CACHE is not None:
        return _NC_CACHE

    nc = bacc.Bacc("TRN2", target_bir_lowering=False, debug=False, num_devices=NCORES)

    x_d = nc.dram_tensor("x_shard", [BLOC, L, D], _F16, kind="ExternalInput")
    ma_d = nc.dram_tensor("ma_shard", [BLOC, L, D], _F16, kind="ExternalOutput")
    res_d = nc.dram_tensor("res_shard", [BLOC, L, D], _F16, kind="ExternalOutput")

    wf_np, wl_np, wh_np = _weights()
    wf_d = nc.inline_tensor(wf_np, name="w_first")
    wl_d = nc.inline_tensor(wl_np, name="w_low")
    wh_d = nc.inline_tensor(wh_np, name="w_high")

    xa, maa, ra = x_d.ap(), ma_d.ap(), res_d.ap()

    with tile.TileContext(nc) as tc:
        with (
            tc.tile_pool(name="consts", bufs=1) as consts,
            tc.tile_pool(name="xpool", bufs=3) as xpool,
            tc.tile_pool(name="mapool", bufs=2) as mapool,
            tc.tile_pool(name="respool", bufs=2) as respool,
            tc.tile_pool(name="pspool", bufs=8, space=bass.MemorySpace.PSUM) as pspool,
        ):
            wf = consts.tile([T, T], _F16, tag="wf")
            wl = consts.tile([T, T], _F16, tag="wl")
            wh = consts.tile([T, T], _F16, tag="wh")
            nc.gpsimd.dma_start(wf[:], wf_d.ap())
            nc.gpsimd.dma_start(wl[:], wl_d.ap())
            nc.gpsimd.dma_start(wh[:], wh_d.ap())

            xg_prev = None
            for b in range(BLOC):
                for g in range(NG):
                    l0 = g * LG
                    xg = xpool.tile([T, G, D], _F16, tag="xg", name=f"xg_{b}_{g}")
                    src = xa[b, l0 : l0 + LG, :].rearrange("(n p) d -> p n d", p=T)
                    nc.sync.dma_start(xg[:], src)

                    mag = mapool.tile([T, G, D], _F16, tag="mag", name=f"mag_{b}_{g}")
                    resg = respool.tile(
                        [T, G, D], _F16, tag="resg", name=f"resg_{b}_{g}"
                    )
                    for n in range(G):
                        ps = pspool.tile([T, D], _F32, tag="ps", name=f"ps_{b}_{g}_{n}")
                        cur = xg[:, n, :]
                        if g == 0 and n == 0:
                            nc.tensor.matmul(ps[:], wf[:], cur, start=True, stop=True)
                        else:
                            prev = (
                                xg[:, n - 1, :]
                                if n > 0
                                else xg_prev[:, G - 1, :]
                            )
                            nc.tensor.matmul(ps[:], wl[:], cur, start=True, stop=False)
                            nc.tensor.matmul(
                                ps[:], wh[:], prev, start=False, stop=True
                            )
                        nc.scalar.copy(mag[:, n, :], ps[:])
                        nc.vector.tensor_sub(resg[:, n, :], cur, ps[:])

                    dst_ma = maa[b, l0 : l0 + LG, :].rearrange("(n p) d -> p n d", p=T)
                    dst_res = ra[b, l0 : l0 + LG, :].rearrange("(n p) d -> p n d", p=T)
                    nc.gpsimd.dma_start(dst_ma, mag[:])
                    nc.gpsimd.dma_start(dst_res, resg[:])
                    xg_prev = xg

    nc.compile()
    _NC_CACHE = nc
    return nc


def make_in_maps(x):
    x16 = np.ascontiguousarray(x, dtype=np.float16)
    return [{"x_shard": x16[c * BLOC : (c + 1) * BLOC]} for c in range(NCORES)]


def kernel(**inputs):
    x = inputs["x"]
    assert x.shape == (B, L, D), x.shape

    nc = build()
    in_maps = make_in_maps(x)
    r = run_bass_kernel_spmd(nc, in_maps, core_ids=list(range(NCORES)))

    res = np.concatenate(
        [r.results[c]["res_shard"] for c in range(NCORES)], axis=0
    ).astype(np.float32)
    ma = np.concatenate(
        [r.results[c]["ma_shard"] for c in range(NCORES)], axis=0
    ).astype(np.float32)
    return (res, ma)


# revision 15
# speedup vs baseline: 1.0366x; 1.0366x over previous
"""Trainium2 Bass kernel for EMA-along-L + residual, x: (32, 4096, 512) fp32.

Blocked-matmul EMA formulation, fp16 I/O.

With alpha=0.3 (beta=0.7), beta^128 ~ 1.6e-20, so the EMA state at step t
depends only on the last <=256 inputs to far below fp32 precision. For a
128-row L-chunk X_c (layout [l=partition, d=free] -- the natural, cheap DMA
layout), the whole chunk's EMA is

    ma_c = M_low @ X_c + M_high @ X_{c-1}

with constant 128x128 matrices M_low[t,j] = alpha*beta^(t-j) (j<=t) and
M_high[t,j] = alpha*beta^(t+128-j); the first chunk of each batch instead
uses M_first (column 0 replaced by beta^t to match the s_0 = x_0 seed).
That is two PE matmuls per chunk with constant stationary weights, no
transposes and no serial carry chain at all.

I/O runs in fp16 (inputs downcast / outputs upcast on host): halves HBM
traffic vs fp32; end-to-end error ~6e-4 vs the 2e-2 gate. Per chunk, ACT
casts PSUM->fp16 ma tile and DVE computes res = x - ma (fp16 out). Loads
are issued on the Sync queue, stores on the GpSimd queue, so input
prefetch never queues behind a store's semaphore wait.

Sharding: batch dim (32) split 4-per-core across 8 NeuronCores; the scan
dim L stays on-core, so no cross-device communication.
"""

import sys

import numpy as np

try:
    import concourse.bass as bass  # noqa: F401
except ImportError:
    sys.path.insert(0, "/opt/trn_rl_repo")

import concourse.bacc as bacc
import concourse.bass as bass
import concourse.mybir as mybir
import concourse.tile as tile
from concourse.bass_utils import run_bass_kernel_spmd

ALPHA = 0.3
BETA = 0.7

B, L, D = 32, 4096, 512
NCORES = 8
BLOC = B // NCORES  # 4 batches per core
T = 128  # chunk rows (PE contraction size)
G = 4  # chunks per DMA group
LG = T * G  # 512 rows per group
NG = L // LG  # 8 groups per batch
H = 2  # chunks per output store (half group)

_F16 = mybir.dt.float16
_F32 = mybir.dt.float32


def _weights():
    """W_first/W_low/W_high, pre-transposed to [j, t] for the PE lhsT slot."""
    ti = np.arange(T)
    p = ti[:, None] - ti[None, :]
    with np.errstate(under="ignore"):
        m_low = np.where(p >= 0, ALPHA * BETA ** np.clip(p, 0, None), 0.0)
        m_first = m_low.copy()
        m_first[:, 0] = BETA**ti
        m_high = ALPHA * BETA ** (ti[:, None] + T - ti[None, :])
    return (
        np.ascontiguousarray(m_first.T).astype(np.float16),
        np.ascontiguousarray(m_low.T).astype(np.float16),
        np.ascontiguousarray(m_high.T).astype(np.float16),
    )


_NC_CACHE = None


def build():
    global _NC_CACHE
    if _NC_CACHE is not None:
        return _NC_CACHE

    nc = bacc.Bacc("TRN2", target_bir_lowering=False, debug=False, num_devices=NCORES)

    x_d = nc.dram_tensor("x_shard", [BLOC, L, D], _F16, kind="ExternalInput")
    ma_d = nc.dram_tensor("ma_shard", [BLOC, L, D], _F16, kind="ExternalOutput")
    res_d = nc.dram_tensor("res_shard", [BLOC, L, D], _F16, kind="ExternalOutput")

    wf_np, wl_np, wh_np = _weights()
    wf_d = nc.inline_tensor(wf_np, name="w_first")
    wl_d = nc.inline_tensor(wl_np, name="w_low")
    wh_d = nc.inline_tensor(wh_np, name="w_high")

    xa, maa, ra = x_d.ap(), ma_d.ap(), res_d.ap()

    with tile.TileContext(nc) as tc:
        with (
            tc.tile_pool(name="consts", bufs=1) as consts,
            tc.tile_pool(name="xpool", bufs=6) as xpool,
            tc.tile_pool(name="mapool", bufs=4) as mapool,
            tc.tile_pool(name="respool", bufs=4) as respool,
            tc.tile_pool(name="pspool", bufs=8, space=bass.MemorySpace.PSUM) as pspool,
        ):
            wf = consts.tile([T, T], _F16, tag="wf")
            wl = consts.tile([T, T], _F16, tag="wl")
            wh = consts.tile([T, T], _F16, tag="wh")
            nc.gpsimd.dma_start(wf[:], wf_d.ap())
            nc.gpsimd.dma_start(wl[:], wl_d.ap())
            nc.gpsimd.dma_start(wh[:], wh_d.ap())

            xg_prev = None
            for b in range(BLOC):
                for g in range(NG):
                    l0 = g * LG
                    xg = xpool.tile([T, G, D], _F16, tag="xg", name=f"xg_{b}_{g}")
                    src = xa[b, l0 : l0 + LG, :].rearrange("(n p) d -> p n d", p=T)
                    nc.sync.dma_start(xg[:], src)

                    mag = mapool.tile([T, G, D], _F16, tag="mag", name=f"mag_{b}_{g}")
                    resg = respool.tile(
                        [T, G, D], _F16, tag="resg", name=f"resg_{b}_{g}"
                    )
                    for n in range(G):
                        ps = pspool.tile([T, D], _F32, tag="ps", name=f"ps_{b}_{g}_{n}")
                        cur = xg[:, n, :]
                        if g == 0 and n == 0:
                            nc.tensor.matmul(ps[:], wf[:], cur, start=True, stop=True)
                        else:
                            prev = (
                                xg[:, n - 1, :]
                                if n > 0
                                else xg_prev[:, G - 1, :]
                            )
                            nc.tensor.matmul(ps[:], wl[:], cur, start=True, stop=False)
                            nc.tensor.matmul(
                                ps[:], wh[:], prev, start=False, stop=True
                            )
                        nc.scalar.copy(mag[:, n, :], ps[:])
                        nc.vector.tensor_sub(resg[:, n, :], cur, ps[:])

                    dst_ma = maa[b, l0 : l0 + LG, :].rearrange("(n p) d -> p n d", p=T)
                    dst_res = ra[b, l0 : l0 + LG, :].rearrange("(n p) d -> p n d", p=T)
                    nc.gpsimd.dma_start(dst_ma, mag[:])
                    nc.gpsimd.dma_start(dst_res, resg[:])
                    xg_prev = xg

    nc.compile()
    _NC_CACHE = nc
    return nc


def make_in_maps(x):
    x16 = np.ascontiguousarray(x, dtype=np.float16)
    return [{"x_shard": x16[c * BLOC : (c + 1) * BLOC]} for c in range(NCORES)]


def kernel(**inputs):
    x = inputs["x"]
    assert x.shape == (B, L, D), x.shape

    nc = build()
    in_maps = make_in_maps(x)
    r = run_bass_kernel_spmd(nc, in_maps, core_ids=list(range(NCORES)))

    res = np.concatenate(
        [r.results[c]["res_shard"] for c in range(NCORES)], axis=0
    ).astype(np.float32)
    ma = np.concatenate(
        [r.results[c]["ma_shard"] for c in range(NCORES)], axis=0
    ).astype(np.float32)
    return (res, ma)
